# revision 19
# baseline (speedup 1.0000x reference)
"""Trainium2 Bass kernel: 16-head attention with RoPE (dense_transformer).

Sharding: tensor-parallel over heads. 8 cores x 2 heads each.
Each core: Wq/Wk/Wv column slice [1024,128], Wo row slice [128,1024],
full input; computes its heads' attention + partial output projection.
Host sums the 8 partial outputs (row-parallel Wo reduction) and adds bo.

Device layout is "transposed": Q^T/K^T/ctx^T are kept as [dim, seq] with
the head dim on SBUF partitions, so Q^T = Wq^T @ X^T comes straight out
of the PE, scores^T = K^T.T @ Q^T needs no transposes, and the softmax
denominator falls out of an extra ones-column appended to V.

v5 schedule: the kernel is paced by the softmax exp (only ACT has an exp
LUT; 128 FD=1024 exps ~ 147us). v5 shrinks the lead-in and tail around
that fixed exp stream:
 - xt arrives POSITION-major (8 chunks of 512 positions x all channels),
   so Q/K projection + rope of the first stripe complete right after
   chunk 0 lands; the first exp fires ~17us instead of ~65us.
 - rope is per-512-stripe with the bias folded into DVE
   scalar_tensor_tensor ((ps+b)*cos from PSUM); the swap half reads
   PSUM via 4 small DMAs and applies a swapped bias vector. ACT does
   NOTHING but exp.
 - softmax division is per-stripe: reciprocal of the PSUM den row,
   gpsimd partition_broadcast (no DRAM bounce), one mul per head half;
   out-projection quads follow per-stripe so the output DMA streams
   throughout instead of bunching at the end.
 - everything else (V proj, batch-1 Q/K+rope, ctx chains, out-proj)
   rides the exp shadow as pump units, as in v4.
"""

import sys

if "/opt/trn_rl_repo" not in sys.path:
    sys.path.insert(0, "/opt/trn_rl_repo")

from collections import deque

import numpy as np
import ml_dtypes

B = 2
S = 2048
NS = B * S  # 4096
D = 1024
H = 16
DK = 64
NCORES = 8
HPC = H // NCORES  # heads per core = 2
DPC = HPC * DK  # model dims per core = 128
NST = NS // 512  # 8 output stripes

_cache = {}


def _build_nc():
    import concourse.bass as bass
    import concourse.tile as tile
    import concourse.mybir as mybir
    from concourse import bacc

    fp32 = mybir.dt.float32
    bf16 = mybir.dt.bfloat16
    Exp = mybir.ActivationFunctionType.Exp
    Add = mybir.AluOpType.add
    Mult = mybir.AluOpType.mult

    nc = bacc.Bacc("TRN2", debug=False, num_devices=NCORES)

    xt = nc.dram_tensor("xt", [D, NS], bf16, kind="ExternalInput").ap()
    wq = nc.dram_tensor("wq", [128, 8 * 128], bf16, kind="ExternalInput").ap()
    wk = nc.dram_tensor("wk", [128, 8 * 128], bf16, kind="ExternalInput").ap()
    wv = nc.dram_tensor("wv", [128, 8 * 128], bf16, kind="ExternalInput").ap()
    wo = nc.dram_tensor("wo", [DPC, D], bf16, kind="ExternalInput").ap()
    bq = nc.dram_tensor("bq", [DPC, 1], fp32, kind="ExternalInput").ap()
    bk = nc.dram_tensor("bk", [DPC, 1], fp32, kind="ExternalInput").ap()
    bv = nc.dram_tensor("bv", [1, DPC], bf16, kind="ExternalInput").ap()
    cos_d = nc.dram_tensor("cos", [128, S], bf16, kind="ExternalInput").ap()
    sin_d = nc.dram_tensor("sin", [128, S], bf16, kind="ExternalInput").ap()
    out_d = nc.dram_tensor("out", [D, NS], bf16, kind="ExternalOutput").ap()

    with tile.TileContext(nc) as tc:
        with (
            tc.tile_pool(name="persist", bufs=1) as persist,
            tc.tile_pool(name="stage", bufs=1) as stage,
            tc.tile_pool(name="sc_ps", bufs=2, space="PSUM") as sc_ps,
            tc.tile_pool(name="ctx_ps", bufs=2, space="PSUM") as ctx_ps,
            tc.tile_pool(name="op_ps", bufs=2, space="PSUM") as op_ps,
        ):
            qrot = persist.tile([128, NS], bf16, tag="qrot")
            krot = persist.tile([128, NS], bf16, tag="krot")
            # v_sb[:, tt, 128h] = 1.0 (denominator column -> PSUM row 0),
            # v_sb[:, tt, 128h+1 : +64] = 0 (dead), 128h+64 : +128 = V dk
            # rows. The 64-row gap lets a stream_shuffle (64 partitions,
            # 64-aligned bases) move ctx rows 64:128 down to ctxT rows
            # 0:64 while den sits at partition 0 (partition_broadcast and
            # shuffle both need aligned bases).
            VC = 128
            v_sb = persist.tile([128, 32, 2 * VC], bf16, tag="v")
            ctxT = persist.tile([128, NS], bf16, tag="ctxT")
            wo_sb = persist.tile([128, 8, 128], bf16, tag="wo")
            cos_sb = persist.tile([128, S], bf16, tag="cos")
            sin_sb = persist.tile([128, S], bf16, tag="sin")
            wq_sb = persist.tile([128, 8, 128], bf16, tag="wq")
            wk_sb = persist.tile([128, 8, 128], bf16, tag="wk")
            wv_sb = persist.tile([128, 8, 128], bf16, tag="wv")
            bq_sb = persist.tile([128, 1], fp32, tag="bq")
            bk_sb = persist.tile([128, 1], fp32, tag="bk")
            bvb = persist.tile([128, 128], bf16, tag="bvb")
            xt_sb = persist.tile([128, 8, NS], bf16, tag="xt")
            xt_r = xt.rearrange("(c p) s -> p c s", p=128)
            out_r = out_d.rearrange("(j p) s -> p j s", p=128)

            # --- DMA prologue -------------------------------------------
            # urgent small stuff on scalar queue (ACT idle now), weights
            # on sync/gpsimd, xt position-chunks alternate sync/gpsimd.
            nc.sync.dma_start(wq_sb[:], wq.rearrange("p (c m) -> p c m", m=128))
            nc.gpsimd.dma_start(wk_sb[:], wk.rearrange("p (c m) -> p c m", m=128))
            nc.scalar.dma_start(bq_sb[:], bq)
            nc.scalar.dma_start(bk_sb[:], bk)
            nc.scalar.dma_start(cos_sb[:], cos_d)
            nc.scalar.dma_start(sin_sb[:], sin_d)
            for s in range(8):
                eng = nc.sync if s % 2 == 0 else nc.gpsimd
                eng.dma_start(
                    xt_sb[:, :, s * 512 : (s + 1) * 512],
                    xt_r[:, :, s * 512 : (s + 1) * 512],
                )
            nc.scalar.dma_start(bvb[:], bv.to_broadcast((128, 128)))
            nc.scalar.dma_start(wv_sb[:], wv.rearrange("p (c m) -> p c m", m=128))
            nc.scalar.dma_start(wo_sb[:], wo.rearrange("p (j m) -> p j m", m=128))

            v_r = v_sb[:].rearrange("p t (h x) -> p t h x", x=VC)
            nc.vector.memset(v_r[:, :, :, 0:1], 1.0)
            nc.vector.memset(v_r[:, :, :, 1:64], 0.0)

            # --- stripe-level units -------------------------------------
            def proj_stripe(w_sb, s, name):
                ps = op_ps.tile([128, 512], fp32, tag="op", name=f"ps_{name}{s}")
                for ch in range(8):
                    nc.tensor.matmul(
                        ps[:],
                        w_sb[:, ch, :],
                        xt_sb[:, ch, s * 512 : (s + 1) * 512],
                        start=(ch == 0),
                        stop=(ch == 7),
                    )
                return ps

            def rope_stripe(ps, b_sb, s, dst, qa, qb):
                # dst[:, sl] = plain*cos + swap(plain)*sin, plain = ps + b.
                # term1 fused from PSUM via STT; swap staged via plain SBUF
                # copy (DMA cannot read PSUM).
                sl = slice(s * 512, (s + 1) * 512)
                so = (s % 4) * 512
                cs = cos_sb[:, so : so + 512]
                sn = sin_sb[:, so : so + 512]
                plain = stage.tile(
                    [128, 512], bf16, tag="plain", bufs=2, name=f"pl{s}"
                )
                nc.vector.tensor_scalar_add(plain[:], ps[:], b_sb[:])
                nc.vector.scalar_tensor_tensor(dst[:, sl], ps[:], b_sb[:], cs, Add, Mult)
                swap = stage.tile(
                    [128, 512], bf16, tag="swap", bufs=2, name=f"sw{s}"
                )
                for g in (0, 64):
                    qa.dma_start(swap[g : g + 32, :], plain[g + 32 : g + 64, :])
                    qb.dma_start(swap[g + 32 : g + 64, :], plain[g : g + 32, :])
                t2 = stage.tile([128, 512], bf16, tag="t2", bufs=2, name=f"t2{s}")
                nc.vector.tensor_mul(t2[:], swap[:], sn)
                nc.vector.tensor_add(dst[:, sl], dst[:, sl], t2[:])

            def qk_unit(w_sb, b_sb, s, dst, name, qa, qb):
                def unit():
                    ps = proj_stripe(w_sb, s, name)
                    rope_stripe(ps, b_sb, s, dst, qa, qb)

                return unit

            def v_chain(tt):
                def unit():
                    psv = op_ps.tile([128, 128], fp32, tag="op", name=f"psv{tt}")
                    for ch in range(8):
                        nc.tensor.matmul(
                            psv[:],
                            xt_sb[:, ch, tt * 128 : (tt + 1) * 128],
                            wv_sb[:, ch, :],
                            start=(ch == 0),
                            stop=(ch == 7),
                        )
                    dst = v_sb[:, tt].rearrange("p (h x) -> p h x", h=2)[
                        :, :, 64 : 64 + DK
                    ]
                    nc.vector.tensor_add(dst, psv[:], bvb[:])

                return unit

            # --- attention block ----------------------------------------
            work = deque()

            def pump(n):
                for _ in range(n):
                    if work:
                        work.popleft()()

            rds = {}

            def ctx_units(b, sh, si, expS):
                # two 16-matmul ctx chains (head 0 / head 1) for the
                # 512-col stripe, as 8 pump units of 4 MMs + evacuation.
                st_i = sh * 2 + si
                st = b * 4 + st_i
                pcs = {}

                def chain_quarter(h, q):
                    def unit():
                        if q == 0:
                            pcs[h] = ctx_ps.tile(
                                [VC, 512], fp32, tag="pc",
                                name=f"pc{b}{st_i}{h}",
                            )
                        for tt in range(q * 4, q * 4 + 4):
                            nc.tensor.matmul(
                                pcs[h][:],
                                v_sb[:, b * 16 + tt, h * VC : (h + 1) * VC],
                                expS[:, tt, h * 512 : (h + 1) * 512],
                                start=(tt == 0),
                                stop=(tt == 15),
                            )

                    return unit

                def evacuate():
                    # pc rows: 0 = den, 64:128 = ctx. bf16 staging copy
                    # (lane-locked, shuffle needs same dtype), then an
                    # identity shuffle shifts ctx down 64 partitions.
                    ds0 = b * S + st_i * 512
                    rr = stage.tile(
                        [1, 2, 512], bf16, tag="rr", bufs=2, name=f"rr{st}"
                    )
                    rds[st] = rr
                    dsb = stage.tile(
                        [1, 2, 512], bf16, tag="dsb", bufs=1, name="dsb"
                    )
                    stg = stage.tile(
                        [128, 2, 512], bf16, tag="stg", bufs=1, name="stg"
                    )
                    nc.vector.tensor_copy(dsb[0:1, 0, :], pcs[0][0:1, :])
                    nc.vector.tensor_copy(
                        stg[64:128, 0, :], pcs[0][64 : 64 + DK, :]
                    )
                    nc.vector.stream_shuffle(
                        ctxT[0:DK, ds0 : ds0 + 512],
                        stg[64:128, 0, :],
                        mask=list(range(32)),
                    )
                    nc.vector.tensor_copy(dsb[0:1, 1, :], pcs[1][0:1, :])
                    nc.vector.tensor_copy(
                        stg[64:128, 1, :], pcs[1][64 : 64 + DK, :]
                    )
                    nc.vector.stream_shuffle(
                        ctxT[DK : 2 * DK, ds0 : ds0 + 512],
                        stg[64:128, 1, :],
                        mask=list(range(32)),
                    )
                    with nc.allow_low_precision(
                        reason="bf16 softmax reciprocal within tolerance"
                    ):
                        nc.vector.reciprocal(rr[0:1, 0, :], dsb[0:1, 0, :])
                        nc.vector.reciprocal(rr[0:1, 1, :], dsb[0:1, 1, :])

                units = []
                for q in range(4):
                    units.append(chain_quarter(0, q))
                    units.append(chain_quarter(1, q))
                units.append(evacuate)
                return units

            def div_unit(st):
                # broadcast the reciprocal rows, scale the ctxT stripe
                def unit():
                    sl = slice(st * 512, (st + 1) * 512)
                    rr = rds.pop(st)
                    R0 = stage.tile([64, 512], bf16, tag="R0", bufs=1, name=f"R0_{st}")
                    R1 = stage.tile([128, 512], bf16, tag="R1", bufs=1, name=f"R1_{st}")
                    nc.gpsimd.partition_broadcast(
                        R0[:], rr[0:1, 0, :], channels=64
                    )
                    nc.gpsimd.partition_broadcast(
                        R1[:], rr[0:1, 1, :], channels=128
                    )
                    nc.vector.tensor_mul(ctxT[0:DK, sl], ctxT[0:DK, sl], R0[:])
                    nc.vector.tensor_mul(
                        ctxT[DK:128, sl], ctxT[DK:128, sl], R1[DK:128, :]
                    )

                return unit

            def op_quad(st, j):
                # 2 out-proj tiles (oc = 2j, 2j+1) -> one 128KB DMA
                def unit():
                    ob = stage.tile(
                        [128, 2, 512], bf16, tag="ob", bufs=2, name=f"ob{st}_{j}"
                    )
                    for k in range(2):
                        oc = j * 2 + k
                        po = op_ps.tile(
                            [128, 512], fp32, tag="op", name=f"po{st}_{oc}"
                        )
                        nc.tensor.matmul(
                            po[:],
                            wo_sb[:, oc, :],
                            ctxT[:, st * 512 : (st + 1) * 512],
                            start=True,
                            stop=True,
                        )
                        nc.vector.tensor_copy(ob[:, k, :], po[:])
                    dq = nc.sync if (st + j) % 2 == 0 else nc.gpsimd
                    dq.dma_start(
                        out_r[:, j * 2 : j * 2 + 2, st * 512 : (st + 1) * 512],
                        ob[:],
                    )

                return unit

            def attn_block(b, sh, si, budgets):
                # both heads' scores into the two banks of one [128,1024]
                # PSUM tile (concurrent PE row-group tiles (0,0)/(64,0));
                # ONE FD=1024 exp covers both heads.
                expS = stage.tile(
                    [128, 16, 1024], bf16, tag="expS", bufs=2,
                    name=f"eS{b}{sh}{si}",
                )
                s0 = b * S + sh * 1024 + si * 512
                for tt in range(16):
                    pump(budgets[tt])
                    tb = slice(b * S + tt * 128, b * S + (tt + 1) * 128)
                    ps = sc_ps.tile([128, 1024], fp32, tag="sc", name="psAB")
                    nc.tensor.matmul(
                        ps[:, 0:512], krot[0:DK, tb], qrot[0:DK, s0 : s0 + 512],
                        start=True, stop=True,
                    )
                    nc.tensor.matmul(
                        ps[:, 512:1024],
                        krot[DK:128, tb],
                        qrot[DK:128, s0 : s0 + 512],
                        start=True, stop=True,
                    )
                    nc.scalar.activation(expS[:, tt, :], ps[:], Exp, scale=0.125)
                return expS

            # --- lead-in: stripe 0 K and Q, explicit --------------------
            ps_k0 = proj_stripe(wk_sb, 0, "k")
            rope_stripe(ps_k0, bk_sb, 0, krot, nc.sync, nc.gpsimd)
            ps_q0 = proj_stripe(wq_sb, 0, "q")
            rope_stripe(ps_q0, bq_sb, 0, qrot, nc.sync, nc.gpsimd)

            # pump inventory. Emission-order constraints (deps must be
            # emitted before their consumers, not just scheduled):
            #  - ropeK1-3 before block0's tt4/8/12 score emission
            #  - ropeQ{n} before block n's first score emission
            #  - v_chain(0..15) before ctx(block0) units (extendleft at bi=1)
            #  - v_chain(16..31) before ctx(block4) units (bi=5)
            # Block 0 runs budget 1 for tt0-5 (K stripes pace the DMA),
            # then 2, so K1-3/Q1 ropes + all 16 b0 V chains drain inside
            # block 0. div/op units are appended as blocks complete.
            for s in (1, 2, 3):
                work.append(qk_unit(wk_sb, bk_sb, s, krot, "k", nc.sync, nc.gpsimd))
            work.append(qk_unit(wq_sb, bq_sb, 1, qrot, "q", nc.sync, nc.gpsimd))
            for tt in range(16):
                work.append(v_chain(tt))
            for s in (2, 3):
                work.append(qk_unit(wq_sb, bq_sb, s, qrot, "q", nc.sync, nc.gpsimd))
            for s in (4, 5, 6, 7):
                work.append(qk_unit(wk_sb, bk_sb, s, krot, "k", nc.sync, nc.gpsimd))
            work.append(qk_unit(wq_sb, bq_sb, 4, qrot, "q", nc.sync, nc.gpsimd))
            for tt in range(16, 32):
                work.append(v_chain(tt))
            for s in (5, 6, 7):
                work.append(qk_unit(wq_sb, bq_sb, s, qrot, "q", nc.sync, nc.gpsimd))

            blocks = [
                (b, sh, si) for b in range(B) for sh in range(2) for si in range(2)
            ]
            b0_budgets = [1] * 6 + [2] * 10
            prev = None
            for bi, (b, sh, si) in enumerate(blocks):
                if prev is not None:
                    work.extendleft(reversed(ctx_units(*prev)))
                expS = attn_block(b, sh, si, b0_budgets if bi == 0 else [2] * 16)
                if bi >= 2:
                    # div+op for the block whose ctx rode block bi-1
                    pst = blocks[bi - 2]
                    stq = pst[0] * 4 + pst[1] * 2 + pst[2]
                    work.append(div_unit(stq))
                    for j in range(4):
                        work.append(op_quad(stq, j))
                prev = (b, sh, si, expS)

            # drain: last block's ctx, remaining pump work, last stripes
            for u in ctx_units(*prev):
                u()
            while work:
                work.popleft()()
            for stq in (6, 7):
                div_unit(stq)()
                for j in range(4):
                    op_quad(stq, j)()

    nc.compile()
    return nc


def _rope_tables():
    pos = np.arange(S, dtype=np.float64)
    inv_freq = np.exp(np.arange(0, DK, 2, dtype=np.float64) * (-np.log(10000.0) / DK))
    ang = pos[:, None] * inv_freq[None, :]  # [S, 32]
    cos_t = np.empty((128, S), dtype=np.float32)
    sin_t = np.empty((128, S), dtype=np.float32)
    c = np.cos(ang).astype(np.float32).T  # [32, S]
    s = np.sin(ang).astype(np.float32).T
    for blk in range(4):
        cos_t[blk * 32 : (blk + 1) * 32] = c
        sign = -1.0 if blk % 2 == 0 else 1.0
        sin_t[blk * 32 : (blk + 1) * 32] = sign * s
    return cos_t, sin_t


def _prep_w(w):
    # [1024, 128] column slice -> [128, 8*128] with the 1024-dim split into
    # 8 chunks of 128 on the partition axis (contiguous 2KB DMA lines)
    bf = ml_dtypes.bfloat16
    return np.ascontiguousarray(
        np.asarray(w, dtype=np.float32)
        .reshape(8, 128, 128)
        .transpose(1, 0, 2)
        .reshape(128, 8 * 128)
    ).astype(bf)


def _prep_inputs(inputs, Wq, bq, Wk, bk, Wv, bv, Wo):
    bf = ml_dtypes.bfloat16
    x2 = np.asarray(inputs, dtype=np.float32).reshape(NS, D)
    xt = np.ascontiguousarray(x2.T).astype(bf)
    cos_t, sin_t = _rope_tables()
    cos_b = cos_t.astype(bf)
    sin_b = sin_t.astype(bf)
    in_maps = []
    for c in range(NCORES):
        sl = slice(c * DPC, (c + 1) * DPC)
        bq_c = np.asarray(bq[sl], dtype=np.float32)
        bk_c = np.asarray(bk[sl], dtype=np.float32)
        in_maps.append(
            {
                "xt": xt,
                "wq": _prep_w(Wq[:, sl]),
                "wk": _prep_w(Wk[:, sl]),
                "wv": _prep_w(Wv[:, sl]),
                "wo": np.ascontiguousarray(Wo[sl, :]).astype(bf),
                "bq": np.ascontiguousarray(bq_c).reshape(DPC, 1),
                "bk": np.ascontiguousarray(bk_c).reshape(DPC, 1),
                "bv": np.ascontiguousarray(bv[sl]).reshape(1, DPC).astype(bf),
                "cos": cos_b,
                "sin": sin_b,
            }
        )
    return in_maps


def _get_nc():
    if "nc" not in _cache:
        _cache["nc"] = _build_nc()
    return _cache["nc"]


def run(inputs_dict, trace=False):
    """Build (cached), run on 8 cores, assemble full output. Returns
    (output fp32 [B,S,D], BassKernelResults)."""
    from concourse.bass_utils import run_bass_kernel_spmd

    nc = _get_nc()
    in_maps = _prep_inputs(
        inputs_dict["inputs"],
        inputs_dict["Wq"],
        inputs_dict["bq"],
        inputs_dict["Wk"],
        inputs_dict["bk"],
        inputs_dict["Wv"],
        inputs_dict["bv"],
        inputs_dict["Wo"],
    )
    res = run_bass_kernel_spmd(
        nc, in_maps, core_ids=list(range(NCORES)), trace=trace
    )
    acc = np.zeros((D, NS), dtype=np.float32)
    for r in res.results:
        acc += r["out"].astype(np.float32)
    out = acc.T.reshape(B, S, D) + np.asarray(inputs_dict["bo"], dtype=np.float32)
    return out.astype(np.float32), res


def kernel(**inputs):
    out, _ = run(inputs, trace=False)
    return out


# revision 26
# speedup vs baseline: 1.0483x; 1.0483x over previous
"""Trainium2 Bass kernel: 16-head attention with RoPE (dense_transformer).

Sharding: tensor-parallel over heads. 8 cores x 2 heads each.
Each core: Wq/Wk/Wv column slice [1024,128], Wo row slice [128,1024],
full input; computes its heads' attention + partial output projection.
Host sums the 8 partial outputs (row-parallel Wo reduction) and adds bo.

Device layout is "transposed": Q^T/K^T/ctx^T are kept as [dim, seq] with
the head dim on SBUF partitions, so Q^T = Wq^T @ X^T comes straight out
of the PE, scores^T = K^T.T @ Q^T needs no transposes, and the softmax
denominator falls out of an extra ones-column in V.

v6 schedule: the kernel is paced by the softmax exp (only ACT has an exp
LUT; 128 FD=1024 exps ~ 147us busy). Everything else is arranged to hide
under that stream:
 - xt arrives POSITION-major (8 chunks of 512 positions x all channels);
   Q/K projection + rope of stripe 0 complete right after chunk 0 lands,
   so the first exp fires ~20us in. Queue plumbing keeps the bulk chunk
   stream (scalar queue) away from the latency-critical rope-swap DMAs
   (sync/gpsimd).
 - rope is per-512-stripe: DVE tensor_scalar_add stages the biased
   plain, scalar_tensor_tensor fuses (ps+b)*cos straight from PSUM,
   swap halves move via 4 small SBUF DMAs. ACT does nothing but exp.
 - V columns per key block: [ones | 0*63 | dk] for head 0 and
   [dk | ones] for head 1, so head0 ctx lands on PSUM rows 64:128 and
   head1 ctx on rows 0:64 - both evacuate with plain lane-aligned
   copies (no stream_shuffle); Wo rows are permuted host-side to match
   ctxT rows = [h1 dk | h0 dk]. Denominators land on PSUM rows 0 (h0)
   and 64 (h1).
 - softmax division per-stripe: dens bounce through DRAM to a [128,8]
   layout (DVE reciprocal is a multi-pass per-lane op - a [1,512] row
   costs 3.3us, [128,8] costs ~0.2us), reciprocal rows return to
   partition 0, gpsimd partition_broadcast fans them out, two DVE muls
   scale the stripe. Out-projection quads follow per-stripe so output
   DMA streams throughout instead of bunching at the end.
"""

import sys

if "/opt/trn_rl_repo" not in sys.path:
    sys.path.insert(0, "/opt/trn_rl_repo")

from collections import deque

import numpy as np
import ml_dtypes

B = 2
S = 2048
NS = B * S  # 4096
D = 1024
H = 16
DK = 64
NCORES = 8
HPC = H // NCORES  # heads per core = 2
DPC = HPC * DK  # model dims per core = 128

_cache = {}


def _build_nc():
    import concourse.bass as bass
    import concourse.tile as tile
    import concourse.mybir as mybir
    from concourse import bacc

    fp32 = mybir.dt.float32
    bf16 = mybir.dt.bfloat16
    Exp = mybir.ActivationFunctionType.Exp
    Add = mybir.AluOpType.add
    Mult = mybir.AluOpType.mult

    nc = bacc.Bacc("TRN2", debug=False, num_devices=NCORES)

    xt = nc.dram_tensor("xt", [D, NS], bf16, kind="ExternalInput").ap()
    wq = nc.dram_tensor("wq", [128, 8 * 128], bf16, kind="ExternalInput").ap()
    wk = nc.dram_tensor("wk", [128, 8 * 128], bf16, kind="ExternalInput").ap()
    wv = nc.dram_tensor("wv", [128, 8 * 128], bf16, kind="ExternalInput").ap()
    wo = nc.dram_tensor("wo", [DPC, D], bf16, kind="ExternalInput").ap()
    bq = nc.dram_tensor("bq", [DPC, 1], fp32, kind="ExternalInput").ap()
    bk = nc.dram_tensor("bk", [DPC, 1], fp32, kind="ExternalInput").ap()
    bv = nc.dram_tensor("bv", [1, DPC], bf16, kind="ExternalInput").ap()
    cos_d = nc.dram_tensor("cos", [128, S], bf16, kind="ExternalInput").ap()
    sin_d = nc.dram_tensor("sin", [128, S], bf16, kind="ExternalInput").ap()
    out_d = nc.dram_tensor("out", [D, NS], bf16, kind="ExternalOutput").ap()

    # v_sb columns per key block tt: [0]=ones(h0), [1:64]=0, [64:128]=h0
    # dk, [128:192]=h1 dk, [192]=ones(h1). h0 lhsT = cols 0:128 (M=128,
    # den->row 0, ctx->rows 64:128); h1 lhsT = cols 128:193 (M=65,
    # ctx->rows 0:64, den->row 64).
    VW = 193

    with tile.TileContext(nc) as tc:
        with (
            tc.tile_pool(name="persist", bufs=1) as persist,
            tc.tile_pool(name="stage", bufs=1) as stage,
            tc.tile_pool(name="dram", bufs=1, space="DRAM") as dram,
            tc.tile_pool(name="sc_ps", bufs=2, space="PSUM") as sc_ps,
            tc.tile_pool(name="ctx_ps", bufs=2, space="PSUM") as ctx_ps,
            tc.tile_pool(name="op_ps", bufs=2, space="PSUM") as op_ps,
        ):
            qrot = persist.tile([128, NS], bf16, tag="qrot")
            krot = persist.tile([128, NS], bf16, tag="krot")
            v_sb = persist.tile([128, 32, VW], bf16, tag="v")
            ctxT = persist.tile([128, NS], bf16, tag="ctxT")
            wo_sb = persist.tile([128, 8, 128], bf16, tag="wo")
            cos_sb = persist.tile([128, S], bf16, tag="cos")
            sin_sb = persist.tile([128, S], bf16, tag="sin")
            wq_sb = persist.tile([128, 8, 128], bf16, tag="wq")
            wk_sb = persist.tile([128, 8, 128], bf16, tag="wk")
            wv_sb = persist.tile([128, 8, 128], bf16, tag="wv")
            bq_sb = persist.tile([128, 1], fp32, tag="bq")
            bk_sb = persist.tile([128, 1], fp32, tag="bk")
            bvb = persist.tile([128, 128], bf16, tag="bvb")
            xt_sb = persist.tile([128, 8, NS], bf16, tag="xt")
            den_dram = dram.tile([8, 2, 512], bf16, tag="den_dram")
            rc_dram = dram.tile([8, 2, 512], bf16, tag="rc_dram")
            xt_r = xt.rearrange("(c p) s -> p c s", p=128)
            out_r = out_d.rearrange("(j p) s -> p j s", p=128)

            def chunk_dma(eng, s):
                eng.dma_start(
                    xt_sb[:, :, s * 512 : (s + 1) * 512],
                    xt_r[:, :, s * 512 : (s + 1) * 512],
                )

            # --- DMA prologue -------------------------------------------
            # scalar queue carries the bulk stream (weights + chunks 4-7)
            # so the latency-critical chunk 0/1 + rope swap DMAs own
            # sync/gpsimd. ACT pays ~0.7us per trigger but is idle until
            # the first exp (~20us), by which time these are all issued.
            nc.scalar.dma_start(wq_sb[:], wq.rearrange("p (c m) -> p c m", m=128))
            nc.scalar.dma_start(bq_sb[:], bq)
            nc.scalar.dma_start(bk_sb[:], bk)
            nc.scalar.dma_start(cos_sb[:], cos_d)
            nc.scalar.dma_start(sin_sb[:], sin_d)
            nc.sync.dma_start(wk_sb[:], wk.rearrange("p (c m) -> p c m", m=128))
            chunk_dma(nc.sync, 0)
            chunk_dma(nc.gpsimd, 1)

            nc.vector.memset(v_sb[:, :, 0:1], 1.0)
            nc.vector.memset(v_sb[:, :, 1:64], 0.0)
            nc.vector.memset(v_sb[:, :, 192:193], 1.0)

            # --- stripe-level units -------------------------------------
            def proj_stripe(w_sb, s, name):
                ps = op_ps.tile([128, 512], fp32, tag="op", name=f"ps_{name}{s}")
                for ch in range(8):
                    nc.tensor.matmul(
                        ps[:],
                        w_sb[:, ch, :],
                        xt_sb[:, ch, s * 512 : (s + 1) * 512],
                        start=(ch == 0),
                        stop=(ch == 7),
                    )
                return ps

            def rope_stripe(ps, b_sb, s, dst, qa, qb):
                # dst[:, sl] = plain*cos + swap(plain)*sin, plain = ps + b.
                # term1 fused from PSUM via STT; swap staged through the
                # biased plain in SBUF (DMA cannot read PSUM).
                sl = slice(s * 512, (s + 1) * 512)
                so = (s % 4) * 512
                cs = cos_sb[:, so : so + 512]
                sn = sin_sb[:, so : so + 512]
                plain = stage.tile(
                    [128, 512], bf16, tag="plain", bufs=2, name=f"pl{s}"
                )
                nc.vector.tensor_scalar_add(plain[:], ps[:], b_sb[:])
                nc.vector.scalar_tensor_tensor(dst[:, sl], ps[:], b_sb[:], cs, Add, Mult)
                swap = stage.tile(
                    [128, 512], bf16, tag="swap", bufs=2, name=f"sw{s}"
                )
                for g in (0, 64):
                    qa.dma_start(swap[g : g + 32, :], plain[g + 32 : g + 64, :])
                    qb.dma_start(swap[g + 32 : g + 64, :], plain[g : g + 32, :])
                t2 = stage.tile([128, 512], bf16, tag="t2", bufs=2, name=f"t2{s}")
                nc.vector.tensor_mul(t2[:], swap[:], sn)
                nc.vector.tensor_add(dst[:, sl], dst[:, sl], t2[:])

            def qk_unit(w_sb, b_sb, s, dst, name):
                def unit():
                    ps = proj_stripe(w_sb, s, name)
                    rope_stripe(ps, b_sb, s, dst, nc.sync, nc.gpsimd)

                return unit

            def v_chain(tt):
                def unit():
                    psv = op_ps.tile([128, 128], fp32, tag="op", name=f"psv{tt}")
                    for ch in range(8):
                        nc.tensor.matmul(
                            psv[:],
                            xt_sb[:, ch, tt * 128 : (tt + 1) * 128],
                            wv_sb[:, ch, :],
                            start=(ch == 0),
                            stop=(ch == 7),
                        )
                    # psv = [pos, h0 dk | h1 dk] -> v cols 64:192
                    nc.vector.tensor_add(v_sb[:, tt, 64:192], psv[:], bvb[:])

                return unit

            # --- attention block ----------------------------------------
            work = deque()

            def pump(n):
                for _ in range(n):
                    if work:
                        work.popleft()()

            rds = {}

            def ctx_units(b, sh, si, expS):
                # two 16-matmul ctx chains (head 0 / head 1) for the
                # 512-col stripe, as 8 pump units of 4 MMs + evacuation.
                st_i = sh * 2 + si
                st = b * 4 + st_i
                pcs = {}

                def chain_quarter(h, q):
                    def unit():
                        if q == 0:
                            pcs[h] = ctx_ps.tile(
                                [128 if h == 0 else 65, 512], fp32, tag="pc",
                                name=f"pc{b}{st_i}{h}",
                            )
                        lo = h * 128
                        hi = 128 if h == 0 else 193
                        for tt in range(q * 4, q * 4 + 4):
                            nc.tensor.matmul(
                                pcs[h][:],
                                v_sb[:, b * 16 + tt, lo:hi],
                                expS[:, tt, h * 512 : (h + 1) * 512],
                                start=(tt == 0),
                                stop=(tt == 15),
                            )

                    return unit

                def evacuate():
                    # h0: den row 0, ctx rows 64:128; h1: ctx rows 0:64,
                    # den row 64. All copies lane-aligned.
                    ds0 = b * S + st_i * 512
                    dsb = stage.tile(
                        [128, 512], bf16, tag="dsb", bufs=2, name=f"dsb{st}"
                    )
                    rds[st] = dsb
                    nc.vector.tensor_copy(
                        ctxT[DK:128, ds0 : ds0 + 512], pcs[0][DK:128, :]
                    )
                    nc.vector.tensor_copy(dsb[0:1, :], pcs[0][0:1, :])
                    nc.vector.tensor_copy(
                        ctxT[0:DK, ds0 : ds0 + 512], pcs[1][0:DK, :]
                    )
                    nc.vector.tensor_copy(dsb[DK : DK + 1, :], pcs[1][DK : DK + 1, :])

                units = []
                for q in range(4):
                    units.append(chain_quarter(0, q))
                    units.append(chain_quarter(1, q))
                units.append(evacuate)
                return units

            def div_unit(st):
                # reshape dens to [128,8] through DRAM (DVE reciprocal is
                # per-lane multi-pass: [1,512] costs 3.3us, [128,8] ~0.2),
                # reciprocal, return to partition-0 rows, broadcast, scale.
                def unit():
                    sl = slice(st * 512, (st + 1) * 512)
                    dsb = rds.pop(st)
                    qa = nc.sync if st % 2 == 0 else nc.gpsimd
                    qb = nc.gpsimd if st % 2 == 0 else nc.sync
                    # both den rows (partitions 0 and 64) in one DMA;
                    # DRAM-side APs reshaped to match the SBUF
                    # partition-first layout.
                    qa.dma_start(
                        den_dram[st].rearrange("h (o f) -> h o f", o=1),
                        dsb[:].rearrange("(a g) f -> a g f", a=2)[:, 0:1, :],
                    )
                    dn = stage.tile([128, 2, 4], bf16, tag="dn", bufs=2, name=f"dn{st}")
                    qa.dma_start(
                        dn[:], den_dram[st].rearrange("h (p j) -> p h j", p=128)
                    )
                    rc = stage.tile([128, 2, 4], bf16, tag="rc", bufs=2, name=f"rc{st}")
                    with nc.allow_low_precision(
                        reason="bf16 softmax reciprocal within tolerance"
                    ):
                        nc.vector.reciprocal(rc[:], dn[:])
                    qb.dma_start(
                        rc_dram[st].rearrange("h (p j) -> p h j", p=128), rc[:]
                    )
                    rrow = stage.tile(
                        [1, 2, 512], bf16, tag="rrow", bufs=2, name=f"rrow{st}"
                    )
                    qb.dma_start(rrow[0:1, :, :], rc_dram[st].rearrange("h (o f) -> o h f", o=1))
                    R0 = stage.tile([64, 512], bf16, tag="R0", bufs=1, name=f"R0_{st}")
                    R1 = stage.tile([128, 512], bf16, tag="R1", bufs=1, name=f"R1_{st}")
                    # ctxT rows 0:64 = h1 (recip row 1), 64:128 = h0 (row 0)
                    nc.gpsimd.partition_broadcast(
                        R0[:], rrow[0:1, 1, :], channels=64
                    )
                    nc.gpsimd.partition_broadcast(
                        R1[:], rrow[0:1, 0, :], channels=128
                    )
                    nc.vector.tensor_mul(ctxT[DK:128, sl], ctxT[DK:128, sl], R1[DK:128, :])
                    nc.vector.tensor_mul(ctxT[0:DK, sl], ctxT[0:DK, sl], R0[:])

                return unit

            def op_quad(st, j):
                # 2 out-proj tiles (oc = 2j, 2j+1) -> one 128KB DMA
                def unit():
                    ob = stage.tile(
                        [128, 2, 512], bf16, tag="ob", bufs=2, name=f"ob{st}_{j}"
                    )
                    for k in range(2):
                        oc = j * 2 + k
                        po = op_ps.tile(
                            [128, 512], fp32, tag="op", name=f"po{st}_{oc}"
                        )
                        nc.tensor.matmul(
                            po[:],
                            wo_sb[:, oc, :],
                            ctxT[:, st * 512 : (st + 1) * 512],
                            start=True,
                            stop=True,
                        )
                        nc.vector.tensor_copy(ob[:, k, :], po[:])
                    dq = nc.sync if (st + j) % 2 == 0 else nc.gpsimd
                    dq.dma_start(
                        out_r[:, j * 2 : j * 2 + 2, st * 512 : (st + 1) * 512],
                        ob[:],
                    )

                return unit

            def attn_block(b, sh, si, budgets):
                # both heads' scores into the two banks of one [128,1024]
                # PSUM tile (concurrent PE row-group tiles (0,0)/(64,0));
                # ONE FD=1024 exp covers both heads.
                expS = stage.tile(
                    [128, 16, 1024], bf16, tag="expS", bufs=2,
                    name=f"eS{b}{sh}{si}",
                )
                s0 = b * S + sh * 1024 + si * 512
                for tt in range(16):
                    pump(budgets[tt])
                    tb = slice(b * S + tt * 128, b * S + (tt + 1) * 128)
                    ps = sc_ps.tile([128, 1024], fp32, tag="sc", name="psAB")
                    nc.tensor.matmul(
                        ps[:, 0:512], krot[0:DK, tb], qrot[0:DK, s0 : s0 + 512],
                        start=True, stop=True,
                    )
                    nc.tensor.matmul(
                        ps[:, 512:1024],
                        krot[DK:128, tb],
                        qrot[DK:128, s0 : s0 + 512],
                        start=True, stop=True,
                    )
                    nc.scalar.activation(expS[:, tt, :], ps[:], Exp, scale=0.125)
                return expS

            # --- lead-in: stripe 0 K and Q, explicit; chunks 2/3 follow
            # the stripe-0 swaps on sync/gpsimd, the rest stream on scalar.
            ps_k0 = proj_stripe(wk_sb, 0, "k")
            rope_stripe(ps_k0, bk_sb, 0, krot, nc.sync, nc.gpsimd)
            chunk_dma(nc.sync, 2)
            ps_q0 = proj_stripe(wq_sb, 0, "q")
            rope_stripe(ps_q0, bq_sb, 0, qrot, nc.sync, nc.gpsimd)
            chunk_dma(nc.gpsimd, 3)
            nc.scalar.dma_start(bvb[:], bv.to_broadcast((128, 128)))
            nc.scalar.dma_start(wv_sb[:], wv.rearrange("p (c m) -> p c m", m=128))
            for s in (4, 5, 6, 7):
                chunk_dma(nc.scalar, s)
            nc.scalar.dma_start(wo_sb[:], wo.rearrange("p (j m) -> p j m", m=128))

            # pump inventory. Emission-order constraints (producers must
            # be emitted before consumers): ropeK1-3 before block0 tt4/8/
            # 12; ropeQn before block n; v(0..15) before ctx(block0)
            # [extendleft at bi=1]; v(16..31) before ctx(block4) [bi=5].
            for s in (1, 2, 3):
                work.append(qk_unit(wk_sb, bk_sb, s, krot, "k"))
            work.append(qk_unit(wq_sb, bq_sb, 1, qrot, "q"))
            for tt in range(16):
                work.append(v_chain(tt))
            for s in (2, 3):
                work.append(qk_unit(wq_sb, bq_sb, s, qrot, "q"))
            for s in (4, 5, 6, 7):
                work.append(qk_unit(wk_sb, bk_sb, s, krot, "k"))
            work.append(qk_unit(wq_sb, bq_sb, 4, qrot, "q"))
            for tt in range(16, 32):
                work.append(v_chain(tt))
            for s in (5, 6, 7):
                work.append(qk_unit(wq_sb, bq_sb, s, qrot, "q"))

            blocks = [
                (b, sh, si) for b in range(B) for sh in range(2) for si in range(2)
            ]
            b0_budgets = [1] * 6 + [2] * 10
            prev = None
            for bi, (b, sh, si) in enumerate(blocks):
                if prev is not None:
                    work.extendleft(reversed(ctx_units(*prev)))
                expS = attn_block(b, sh, si, b0_budgets if bi == 0 else [2] * 16)
                if bi >= 2:
                    # div+op for the block whose ctx rode block bi-1
                    pst = blocks[bi - 2]
                    stq = pst[0] * 4 + pst[1] * 2 + pst[2]
                    work.append(div_unit(stq))
                    for j in range(4):
                        work.append(op_quad(stq, j))
                prev = (b, sh, si, expS)

            # drain: last block's ctx, remaining pump work, last stripes
            for u in ctx_units(*prev):
                u()
            while work:
                work.popleft()()
            for stq in (6, 7):
                div_unit(stq)()
                for j in range(4):
                    op_quad(stq, j)()

    nc.compile()
    return nc


def _rope_tables():
    pos = np.arange(S, dtype=np.float64)
    inv_freq = np.exp(np.arange(0, DK, 2, dtype=np.float64) * (-np.log(10000.0) / DK))
    ang = pos[:, None] * inv_freq[None, :]  # [S, 32]
    cos_t = np.empty((128, S), dtype=np.float32)
    sin_t = np.empty((128, S), dtype=np.float32)
    c = np.cos(ang).astype(np.float32).T  # [32, S]
    s = np.sin(ang).astype(np.float32).T
    for blk in range(4):
        cos_t[blk * 32 : (blk + 1) * 32] = c
        sign = -1.0 if blk % 2 == 0 else 1.0
        sin_t[blk * 32 : (blk + 1) * 32] = sign * s
    return cos_t, sin_t


def _prep_w(w):
    # [1024, 128] column slice -> [128, 8*128] with the 1024-dim split into
    # 8 chunks of 128 on the partition axis (contiguous 2KB DMA lines)
    bf = ml_dtypes.bfloat16
    return np.ascontiguousarray(
        np.asarray(w, dtype=np.float32)
        .reshape(8, 128, 128)
        .transpose(1, 0, 2)
        .reshape(128, 8 * 128)
    ).astype(bf)


def _prep_inputs(inputs, Wq, bq, Wk, bk, Wv, bv, Wo):
    bf = ml_dtypes.bfloat16
    x2 = np.asarray(inputs, dtype=np.float32).reshape(NS, D)
    xt = np.ascontiguousarray(x2.T).astype(bf)
    cos_t, sin_t = _rope_tables()
    cos_b = cos_t.astype(bf)
    sin_b = sin_t.astype(bf)
    in_maps = []
    for c in range(NCORES):
        sl = slice(c * DPC, (c + 1) * DPC)
        # ctxT rows = [h1 dk | h0 dk] -> permute Wo rows to match
        wo_c = np.asarray(Wo[sl, :], dtype=np.float32)
        wo_c = np.concatenate([wo_c[DK:], wo_c[:DK]], axis=0)
        in_maps.append(
            {
                "xt": xt,
                "wq": _prep_w(Wq[:, sl]),
                "wk": _prep_w(Wk[:, sl]),
                "wv": _prep_w(Wv[:, sl]),
                "wo": np.ascontiguousarray(wo_c).astype(bf),
                "bq": np.ascontiguousarray(np.asarray(bq[sl], dtype=np.float32)).reshape(DPC, 1),
                "bk": np.ascontiguousarray(np.asarray(bk[sl], dtype=np.float32)).reshape(DPC, 1),
                "bv": np.ascontiguousarray(bv[sl]).reshape(1, DPC).astype(bf),
                "cos": cos_b,
                "sin": sin_b,
            }
        )
    return in_maps


def _get_nc():
    if "nc" not in _cache:
        _cache["nc"] = _build_nc()
    return _cache["nc"]


def run(inputs_dict, trace=False):
    """Build (cached), run on 8 cores, assemble full output. Returns
    (output fp32 [B,S,D], BassKernelResults)."""
    from concourse.bass_utils import run_bass_kernel_spmd

    nc = _get_nc()
    in_maps = _prep_inputs(
        inputs_dict["inputs"],
        inputs_dict["Wq"],
        inputs_dict["bq"],
        inputs_dict["Wk"],
        inputs_dict["bk"],
        inputs_dict["Wv"],
        inputs_dict["bv"],
        inputs_dict["Wo"],
    )
    res = run_bass_kernel_spmd(
        nc, in_maps, core_ids=list(range(NCORES)), trace=trace
    )
    acc = np.zeros((D, NS), dtype=np.float32)
    for r in res.results:
        acc += r["out"].astype(np.float32)
    out = acc.T.reshape(B, S, D) + np.asarray(inputs_dict["bo"], dtype=np.float32)
    return out.astype(np.float32), res


def kernel(**inputs):
    out, _ = run(inputs, trace=False)
    return out
